# revision 1
# baseline (speedup 1.0000x reference)
"""Trainium2 Bass kernel for the attention-MLP problem.

Reference computation (S=32768, H=1024):
    cat    = [broadcast(hidden, (S, 2H)) | encoder_output]   # [S, 3H]
    energy = tanh(cat @ attn_w.T + attn_b)                   # [S, H]
    logits = (energy @ v_w.T).squeeze()                      # [S]
    out    = softmax(logits)                                 # [S]

Because the hidden rows are identical, cat @ attn_w.T splits into
    c0  = hidden @ W1T + attn_b          (one row, [H])
    pre = enc @ W2T + c0                  (the real work)
with W1T = attn_w[:, :2H].T and W2T = attn_w[:, 2H:].T.

Sharding: seq axis split across 8 cores (4096 rows each); weights
replicated. Softmax normalization uses exp (no max subtraction needed:
|logits| <= ||v_w||_1 ~ 26, safely inside fp32 exp range) with an
AllGather of the 8 per-core partial sums.

Per-core layout (OUT^T): encoder shard is shipped pre-transposed
[H, S_loc] so the H contraction sits on SBUF partitions for both matmul
operands; psum tiles hold energy^T [j, s] so the bias add is a
per-partition ScalarE bias and the v-dot is an accumulating matmul with
vw as the [128,1] stationary operand.
"""

import numpy as np

import concourse.bass as bass
import concourse.mybir as mybir
import concourse.tile as tile
from concourse.bass_utils import run_bass_kernel_spmd

H = 1024
S = 32768
NCORES = 8
SL = S // NCORES          # 4096 rows per core
SB = 512                  # seq block (columns of the psum tiles)
NSB = SL // SB            # 8 seq blocks per core
KC = H // 128             # 8 contraction chunks
JC = H // 128             # 8 output-row chunks

F32 = mybir.dt.float32
F32R = mybir.dt.float32r
BF16 = mybir.dt.bfloat16

AF = mybir.ActivationFunctionType


# ---------------------------------------------------------------------------
# Workaround for this walrus build: instructions only accept a single
# sync-wait command, but Tile can attach several. Hoist the extra waits
# onto NOPs inserted just before the instruction on the same engine
# (engines execute their stream in order, so semantics are preserved).
def _split_multi_waits(nc):
    end_bb = nc.cur_bb.bb
    for bb in nc.m.functions[0].blocks:
        insts = list(bb.instructions)
        out = []
        changed = False
        for inst in insts:
            si = inst.sync_info
            waits = list(si.on_wait) if si and si.on_wait else []
            if len(waits) > 1:
                changed = True
                for w in waits[:-1]:
                    nop = nc.engines[inst.engine].nop(nofuse=True).ins
                    end_bb.instructions.remove(nop)
                    nop.sync_info = mybir.SyncInfo(on_wait=[w], on_update=[])
                    out.append(nop)
                si.on_wait = waits[-1:]
            out.append(inst)
        if changed:
            bb.instructions = out
# ---------------------------------------------------------------------------


def build(repeat: int = 1, main_dt: str = "f32r", mode: str = "full",
          single_core: bool = False, vdot_dt: str = "f32r",
          c0_dt: str = "f32r"):
    """Build the per-core Bass module. `repeat` wraps the main compute in a
    For_i loop (used only by the benchmark harness to measure HW time by
    marginal wall-clock; the softmax tail + collective stay outside).
    mode: full | mm_only | dma_only | full_nodma (perf experiments)."""
    mm_only = mode in ("mm_only", "dma_only")
    full2 = mode == "full2"
    MD = {"f32r": F32R, "bf16": BF16}[main_dt]
    VD = {"f32r": F32R, "bf16": BF16}[vdot_dt]
    CD = {"f32r": F32R, "bf16": BF16}[c0_dt]
    assert not (full2 and vdot_dt != "bf16"), \
        "full2 tile_position v-dot requires bf16"
    nc = bass.Bass("TRN2", target_bir_lowering=False, debug=False,
                   num_devices=1 if single_core else NCORES)

    encT = nc.dram_tensor("encT", [H, SL], MD, kind="ExternalInput").ap()
    w2t = nc.dram_tensor("w2t", [H, H], MD, kind="ExternalInput").ap()
    w1t = nc.dram_tensor("w1t", [2 * H // NCORES, H], CD,
                         kind="ExternalInput").ap()
    hidT = nc.dram_tensor("hidT", [128, 16 // NCORES], CD,
                          kind="ExternalInput").ap()
    bias = nc.dram_tensor("bias", [1, H], F32, kind="ExternalInput").ap()
    vwc = nc.dram_tensor("vwc", [128, JC], VD, kind="ExternalInput").ap()
    out = nc.dram_tensor("out", [1, SL], F32, kind="ExternalOutput").ap()

    encT_v = encT.rearrange("(k p) s -> p k s", p=128)   # [128, 8, 4096]
    w2t_v = w2t.rearrange("(k p) j -> p k j", p=128)     # [128, 8, 1024]
    w1t_v = w1t.rearrange("(k p) j -> p k j", p=128)     # [128, 2, 1024]

    with tile.TileContext(nc) as tc:
        with (
            tc.tile_pool(name="const", bufs=1) as const_pool,
            tc.tile_pool(name="w1", bufs=4) as w1_pool,
            tc.tile_pool(name="enc", bufs=8 if full2 else 4) as enc_pool,
            tc.tile_pool(name="tanh", bufs=4) as tanh_pool,
            tc.tile_pool(name="sm", bufs=1) as sm_pool,
            tc.tile_pool(name="pse", bufs=5 if full2 else 4,
                         space="PSUM") as pse_pool,
            tc.tile_pool(name="psa", bufs=1 if full2 else 2,
                         space="PSUM") as psa_pool,
            tc.tile_pool(name="dram", bufs=1, space="DRAM") as dram_pool,
        ):
            # --- tiny constants -------------------------------------------
            hid_sb = const_pool.tile([128, 16 // NCORES], CD)
            nc.sync.dma_start(hid_sb[:], hidT[:])
            vw_sb = const_pool.tile([128, JC], VD)
            nc.sync.dma_start(vw_sb[:], vwc[:])
            b_sb = const_pool.tile([1, H], F32)
            nc.sync.dma_start(b_sb[:], bias[:])

            # --- replicated weights: one tile per j-slab so the group-j
            # matmuls depend only on their own slab's DMA ---------------
            w2_tiles = []
            for j in range(JC):
                w2_j = const_pool.tile([128, KC, 128], MD, name=f"w2_{j}")
                nc.sync.dma_start(w2_j[:], w2t_v[:, :, j * 128:(j + 1) * 128])
                w2_tiles.append(w2_j)

            exps = sm_pool.tile([1, SL], F32)
            sums = sm_pool.tile([1, NSB], F32)

            # --- c0 = hidden @ W1T + attn_b (one row). Emitted via a
            # closure so the repeat==1 path can place it AFTER s-block 0's
            # matmuls in the PE stream: the PE then crunches real work while
            # the 8MB w1t DMA streams in, instead of idling at the head of
            # its in-order queue.
            c0_sb = const_pool.tile([128, JC], F32)

            psum_c = [pse_pool.tile([1, 512], F32, tag="c0ps", bufs=2,
                                    name=f"psum_c{h}")
                      for h in range(2)]

            NKC = 16 // NCORES   # local w1 chunks (c0 sharded over cores)

            def c0_matmuls(kcs):
                for kc in kcs:
                    w1_t = w1_pool.tile([128, H], CD, tag="w1t", name="w1_t")
                    nc.sync.dma_start(w1_t[:], w1t_v[:, kc, :])
                    for half in range(2):
                        nc.tensor.matmul(
                            psum_c[half][:],
                            hid_sb[:, kc:kc + 1],
                            w1_t[:, half * 512:(half + 1) * 512],
                            start=(kc == 0), stop=(kc == NKC - 1),
                        )

            def c0_finish():
                # bias arrives pre-divided by NCORES, so adding it to the
                # local partial and AllReduce-summing reconstructs c0+b
                part_row = const_pool.tile([1, H], F32)
                for half in range(2):
                    nc.vector.tensor_add(
                        part_row[:, half * 512:(half + 1) * 512],
                        psum_c[half][:],
                        b_sb[:, half * 512:(half + 1) * 512])
                ar_in = dram_pool.tile([1, H], F32)
                nc.gpsimd.dma_start(ar_in[:], part_row[:])
                if single_core:
                    ar_out = ar_in
                else:
                    ar_out = dram_pool.tile([1, H], F32)
                    nc.gpsimd.collective_compute(
                        "AllReduce",
                        mybir.AluOpType.add,
                        replica_groups=[list(range(NCORES))],
                        ins=[ar_in.opt()],
                        outs=[ar_out.opt()],
                    )
                nc.sync.dma_start(
                    c0_sb[:],
                    ar_out[:].rearrange("o (j p) -> (o p) j", p=128)
                )

            def c0_section():
                c0_matmuls(range(NKC))
                c0_finish()

            # --- main pipeline -------------------------------------------
            enc_all = None
            if mode == "full_nodma":
                enc_all = const_pool.tile([128, KC, SL], MD)
                nc.sync.dma_start(enc_all[:], encT_v[:])

            def main_body2(_iv=None):
                HALF = 4
                for half in range(2):
                    enc_ts = []
                    for s in range(HALF):
                        sb = half * HALF + s
                        enc_t = enc_pool.tile([128, KC, SB], MD)
                        nc.sync.dma_start(
                            enc_t[:], encT_v[:, :, sb * SB:(sb + 1) * SB]
                        )
                        enc_ts.append(enc_t)
                    psum_a = psa_pool.tile([128, SB], F32)
                    for j in range(JC):
                        pe = [pse_pool.tile([128, SB], F32, tag="psum_e",
                                            bufs=5, name=f"pe{s}")
                              for s in range(HALF)]
                        for k in range(KC):
                            w = w2_tiles[j][:, k, :]
                            for s in range(HALF):
                                nc.tensor.matmul(
                                    pe[s][:], w, enc_ts[s][:, k, :],
                                    start=(k == 0), stop=(k == KC - 1),
                                )
                        for s in range(HALF):
                            th = tanh_pool.tile([128, SB], VD)
                            nc.scalar.activation(
                                th[:], pe[s][:], AF.Tanh,
                                bias=c0_sb[:, j:j + 1]
                            )
                            nc.tensor.matmul(
                                psum_a[32 * s:32 * s + 1, :],
                                vw_sb[:, j:j + 1], th[:],
                                tile_position=(0, 32 * s),
                                start=(j == 0), stop=(j == JC - 1),
                            )
                    for s in range(HALF):
                        sb = half * HALF + s
                        nc.scalar.activation(
                            exps[:, sb * SB:(sb + 1) * SB],
                            psum_a[32 * s:32 * s + 1, :], AF.Exp,
                            accum_out=sums[:, sb:sb + 1],
                        )

            def load_enc(sb):
                if mode == "full_nodma":
                    return [enc_all[:, k, sb * SB:(sb + 1) * SB]
                            for k in range(KC)]
                enc_t = []
                for k in range(KC):
                    e_k = enc_pool.tile([128, SB], MD, tag=f"enc{k}",
                                        bufs=4, name=f"enc{k}")
                    nc.sync.dma_start(
                        e_k[:], encT_v[:, k, sb * SB:(sb + 1) * SB])
                    enc_t.append(e_k)
                return enc_t

            def mm_group(enc_t, j):
                psum_e = pse_pool.tile([128, SB], F32, tag="psum_e",
                                       name="psum_e")
                for k in range(KC):
                    nc.tensor.matmul(
                        psum_e[:], w2_tiles[j][:, k, :], enc_t[k][:],
                        start=(k == 0), stop=(k == KC - 1),
                    )
                return psum_e

            def consume_group(psum_e, psum_a, j):
                th = tanh_pool.tile([128, SB], VD)
                nc.scalar.activation(
                    th[:], psum_e[:], AF.Tanh, bias=c0_sb[:, j:j + 1])
                nc.tensor.matmul(
                    psum_a[:], vw_sb[:, j:j + 1], th[:],
                    start=(j == 0), stop=(j == JC - 1),
                )

            def sblock_tail(psum_a, sb):
                nc.scalar.activation(
                    exps[:, sb * SB:(sb + 1) * SB], psum_a[:], AF.Exp,
                    accum_out=sums[:, sb:sb + 1],
                )

            def main_body(_iv=None, only_sb=None, prefill=None):
                if full2:
                    return main_body2(_iv)
                for sb in (only_sb if only_sb is not None else range(NSB)):
                    if sb == 0 and prefill is not None:
                        enc_t, groups = prefill
                    else:
                        enc_t, groups = load_enc(sb), []
                    if mode == "dma_only":
                        continue
                    psum_a = None if mm_only else psa_pool.tile([1, SB], F32)
                    for j in range(JC):
                        psum_e = groups[j] if j < len(groups)                             else mm_group(enc_t, j)
                        if mm_only:
                            continue
                        consume_group(psum_e, psum_a, j)
                    if not mm_only:
                        sblock_tail(psum_a, sb)

            c0_section()
            if repeat == 1:
                main_body()
            else:
                with tc.For_i(0, repeat, 1,
                              hint_engines=(mybir.EngineType.PE,)) as _i:
                    main_body(_i)

            # --- softmax normalization across cores -----------------------
            if mm_only:
                nc.gpsimd.memset(exps[:], 1.0)
                nc.gpsimd.memset(sums[:], 1.0)
            if single_core:
                zg = sm_pool.tile([1, 1], F32)
                nc.vector.reduce_sum(zg[:], sums[:],
                                     axis=mybir.AxisListType.X)
            else:
                # AllGather the raw per-block sums (8 floats/core) and do a
                # single 64-element reduce afterwards — one fewer serialized
                # DVE op + drain ahead of the collective.
                ag_in = dram_pool.tile([1, NSB], F32)
                nc.gpsimd.dma_start(ag_in[:], sums[:])
                ag_out = dram_pool.tile([1, NCORES * NSB], F32)
                nc.gpsimd.collective_compute(
                    "AllGather",
                    mybir.AluOpType.bypass,
                    replica_groups=[list(range(NCORES))],
                    ins=[ag_in.opt()],
                    outs=[ag_out.opt()],
                )
                zs = sm_pool.tile([1, NCORES * NSB], F32)
                nc.gpsimd.dma_start(zs[:], ag_out[:])
                zg = sm_pool.tile([1, 1], F32)
                nc.vector.reduce_sum(zg[:], zs[:], axis=mybir.AxisListType.X)
            invz = sm_pool.tile([1, 1], F32)
            nc.vector.reciprocal(invz[:], zg[:])
            outv = sm_pool.tile([1, SL], F32)
            # split the 4096-element scale across ACT and DVE in parallel,
            # and ship each half as soon as it's done
            hl = SL // 2
            nc.scalar.activation(outv[:, :hl], exps[:, :hl], AF.Identity,
                                 scale=invz[:])
            nc.sync.dma_start(out[:, :hl], outv[:, :hl])
            nc.vector.tensor_scalar_mul(outv[:, hl:], exps[:, hl:], invz[:])
            nc.sync.dma_start(out[:, hl:], outv[:, hl:])

    _split_multi_waits(nc)
    return nc


def prepare_in_maps(hidden, encoder_output, attn_w, attn_b, v_w,
                    main_dt="f32r", vdot_dt="f32r", c0_dt="f32r"):
    hidden = np.asarray(hidden, dtype=np.float32)
    enc = np.asarray(encoder_output, dtype=np.float32)
    attn_w = np.asarray(attn_w, dtype=np.float32)
    attn_b = np.asarray(attn_b, dtype=np.float32)
    v_w = np.asarray(v_w, dtype=np.float32)

    import ml_dtypes
    def npdt(s):
        return np.float32 if s == "f32r" else ml_dtypes.bfloat16
    md, vd, cd = npdt(main_dt), npdt(vdot_dt), npdt(c0_dt)
    w2t = np.ascontiguousarray(attn_w[:, 2 * H:].T).astype(md)   # [H, H]
    w1t_full = np.ascontiguousarray(attn_w[:, :2 * H].T).astype(cd)
    hidT_full = np.ascontiguousarray(hidden.reshape(16, 128).T).astype(cd)
    kpc = 16 // NCORES
    b = np.ascontiguousarray(attn_b.reshape(1, H)) / np.float32(NCORES)
    vwc = np.ascontiguousarray(v_w.reshape(JC, 128).T).astype(vd)  # [128, 8]

    in_maps = []
    for c in range(NCORES):
        encT = np.ascontiguousarray(enc[c * SL:(c + 1) * SL, :].T).astype(md)
        in_maps.append({
            "encT": encT, "w2t": w2t,
            "w1t": np.ascontiguousarray(
                w1t_full[c * kpc * 128:(c + 1) * kpc * 128, :]),
            "hidT": np.ascontiguousarray(
                hidT_full[:, c * kpc:(c + 1) * kpc]),
            "bias": b, "vwc": vwc,
        })
    return in_maps


_NC_CACHE = {}


def _get_nc(repeat: int = 1):
    if repeat not in _NC_CACHE:
        _NC_CACHE[repeat] = build(repeat)
    return _NC_CACHE[repeat]


def kernel(hidden, encoder_output, attn_w, attn_b, v_w):
    nc = _get_nc(1)
    in_maps = prepare_in_maps(hidden, encoder_output, attn_w, attn_b, v_w)
    res = run_bass_kernel_spmd(nc, in_maps, list(range(NCORES)))
    return np.concatenate([res.results[c]["out"][0] for c in range(NCORES)])

